# revision 10
# baseline (speedup 1.0000x reference)
"""Trainium2 Bass kernel for nn_ConfounderEncoder.

Computation (full shapes):
    bow[b,v]   = normalized bag-of-words histogram of reports_ids   [256, 8192]
    pmi_vec    = bow @ PMI                                          [256, 8192]
    z_lang     = pmi_vec @ W_lang.T                                 [256, 256]
    h          = gelu(corr.flatten() @ W1.T + b1)                   [1, 512]
    z_corr     = broadcast(h @ W2.T + b2)                           [256, 256]
    out        = concat([z_lang, z_corr], -1)                       [256, 512]

Strategy: PMI (256 MB) is the dominant memory traffic; it is column-sharded
across 8 NeuronCores (each core owns a [8192, 1024] slice, read exactly once).
bow is sent as exact integer token counts (transposed, [8192, 256]) — integers
<= 128 are exact in bf16 — and the 1/count normalization is applied to the
final z_lang partials on the host (z_lang is linear in bow rows).

On each core, matmul #1 uses the streamed PMI tile as the *stationary* operand
so the result comes out transposed (pmi_vecT [v_out, b]) — exactly the layout
matmul #2 (contract v_out against W_langT) needs, avoiding any on-chip
transpose. Each core emits a [256, 256] z_lang partial; the host sums the 8
partials (the cross-core reduction commutes through the linear matmul #2).
The tiny corr MLP runs replicated on every core; core 0's result is used.

Precision modes for the big matmul ("bf16x2" default): PMI is split on the
host into hi = bf16(PMI) and lo = bf16(PMI - hi); both accumulate into the
same fp32 PSUM. Error ~1e-5 relative, at 4x the fp32 PE throughput.
"""

import numpy as np
import ml_dtypes
from contextlib import ExitStack

import concourse.bass as bass
import concourse.mybir as mybir
import concourse.tile as tile
from concourse import bacc
from concourse.bass_utils import run_bass_kernel_spmd
from concourse.masks import make_identity

B, T, V, ZL, ZC, L, H = 256, 128, 8192, 256, 256, 14, 512
LL = L * L            # 196
NCORES = 8
VS = V // NCORES      # 1024 pmi columns per core
KT = V // 128         # 64 k tiles over v_in
MB = VS // 128        # 8 v_out blocks per core

MODE = "bf16x2"       # "f32" | "f32r" | "bf16" | "bf16x2"
MM2_MODE = "f32"      # "f32" | "f32r"

F32 = mybir.dt.float32
BF16 = mybir.dt.bfloat16


def _mode_dtypes(mode):
    if mode in ("bf16", "bf16x2"):
        return BF16, ml_dtypes.bfloat16
    if mode == "f32r":
        return mybir.dt.float32r, np.float32
    return F32, np.float32


def _build_program(mode, mm2_mode, debug_pv=False):
    mm_dt, _ = _mode_dtypes(mode)
    mm2_dt = mybir.dt.float32r if mm2_mode == "f32r" else F32
    two_pass = mode == "bf16x2"

    nc = bacc.Bacc("TRN2", target_bir_lowering=False, debug=False,
                   num_devices=NCORES)

    pmi_hi_d = nc.dram_tensor("pmi_hi", [V, VS], mm_dt, kind="ExternalInput")
    pmi_lo_d = (nc.dram_tensor("pmi_lo", [V, VS], mm_dt, kind="ExternalInput")
                if two_pass else None)
    bowT_d = nc.dram_tensor("bowT", [V, B], mm_dt, kind="ExternalInput")
    wlT_d = nc.dram_tensor("wlT", [VS, ZL], mm2_dt, kind="ExternalInput")
    w1T_d = nc.dram_tensor("w1T", [LL, H], F32, kind="ExternalInput")
    corrT_d = nc.dram_tensor("corrT", [LL, 1], F32, kind="ExternalInput")
    b1_d = nc.dram_tensor("b1c", [128, H // 128], F32, kind="ExternalInput")
    w2T_d = nc.dram_tensor("w2T", [H, ZC], F32, kind="ExternalInput")
    b2_d = nc.dram_tensor("b2c", [128, ZC // 128], F32, kind="ExternalInput")

    zpart_d = nc.dram_tensor("zpart", [B, ZL], F32, kind="ExternalOutput")
    zc_d = nc.dram_tensor("zcT", [ZC, 1], F32, kind="ExternalOutput")
    pv_d = (nc.dram_tensor("pvT", [VS, B], F32, kind="ExternalOutput")
            if debug_pv else None)

    with tile.TileContext(nc) as tc, ExitStack() as ctx:
        const = ctx.enter_context(tc.tile_pool(name="const", bufs=1))

        # ---------------- corr MLP (tiny, replicated) ----------------
        with tc.tile_pool(name="psc", bufs=1, space="PSUM") as psc:
            w1_sb = const.tile([128, 2, H], F32, name="w1_sb")
            nc.gpsimd.dma_start(w1_sb[:, 0, :], w1T_d[0:128, :])
            nc.gpsimd.dma_start(w1_sb[0:LL - 128, 1, :], w1T_d[128:LL, :])
            corr_sb = const.tile([128, 2], F32, name="corr_sb")
            nc.gpsimd.dma_start(corr_sb[:, 0:1], corrT_d[0:128, :])
            nc.gpsimd.dma_start(corr_sb[0:LL - 128, 1:2], corrT_d[128:LL, :])
            b1_sb = const.tile([128, H // 128], F32, name="b1_sb")
            nc.gpsimd.dma_start(b1_sb[:], b1_d[:])
            w2_sb = const.tile([128, H // 128, ZC], F32, name="w2_sb")
            nc.gpsimd.dma_start(
                w2_sb[:], w2T_d.rearrange("(k p) z -> p k z", p=128))
            b2_sb = const.tile([128, ZC // 128], F32, name="b2_sb")
            nc.gpsimd.dma_start(b2_sb[:], b2_d[:])

            h_sb = const.tile([128, H // 128], F32, name="h_sb")
            for m in range(H // 128):
                ph = psc.tile([128, 1], F32, tag="ph", name="ph", bufs=2)
                nc.tensor.matmul(ph[:], w1_sb[:, 0, m * 128:(m + 1) * 128],
                                 corr_sb[:, 0:1], start=True, stop=False)
                nc.tensor.matmul(ph[:],
                                 w1_sb[0:LL - 128, 1, m * 128:(m + 1) * 128],
                                 corr_sb[0:LL - 128, 1:2],
                                 start=False, stop=True)
                nc.scalar.activation(h_sb[:, m:m + 1], ph[:],
                                     mybir.ActivationFunctionType.Gelu,
                                     bias=b1_sb[:, m:m + 1], scale=1.0)

            zc_sb = const.tile([128, ZC // 128], F32, name="zc_sb")
            for m in range(ZC // 128):
                pz = psc.tile([128, 1], F32, tag="pz", name="pz", bufs=2)
                for k in range(H // 128):
                    nc.tensor.matmul(pz[:],
                                     w2_sb[:, k, m * 128:(m + 1) * 128],
                                     h_sb[:, k:k + 1],
                                     start=(k == 0), stop=(k == H // 128 - 1))
                nc.scalar.activation(zc_sb[:, m:m + 1], pz[:],
                                     mybir.ActivationFunctionType.Identity,
                                     bias=b2_sb[:, m:m + 1], scale=1.0)
            nc.gpsimd.dma_start(
                zc_d.rearrange("(m p) one -> p (m one)", p=128), zc_sb[:])

        # ---------------- matmul #1: pmi_vec = bow @ PMI_slice --------------
        # bow (transposed, integer counts) is the stationary operand: each
        # [128,128] lhsT tile is reused across 4-8 N=512 matmuls. PMI streams
        # as the moving operand in KB-k-tile batches (big DMAs), split across
        # the two HWDGE rings (sync / scalar).
        KB = 8                      # k-tiles per DMA batch
        NC2 = VS // 512             # 512-wide v_out chunks per core (2)
        bow_sb = const.tile([128, KT, B], mm_dt, name="bow_sb")
        bow_src = bowT_d.rearrange("(k p) b -> p k b", p=128)
        for c in range(4):
            kc = KT // 4
            nc.gpsimd.dma_start(bow_sb[:, c * kc:(c + 1) * kc, :],
                                bow_src[:, c * kc:(c + 1) * kc, :])

        wl_sb = const.tile([128, MB, ZL], mm2_dt, name="wl_sb")
        nc.gpsimd.dma_start(wl_sb[:],
                            wlT_d.rearrange("(k p) z -> p k z", p=128))
        ident = const.tile([128, 128], F32, name="ident")
        make_identity(nc, ident[:])

        pmi_pool = ctx.enter_context(tc.tile_pool(name="pmi", bufs=1))
        pvec_sb = const.tile([128, B // 128, VS], F32, name="pvec_sb")
        nbufs = 3 if mm_dt == BF16 else 2
        with tc.tile_pool(name="acc", bufs=1, space="PSUM") as acc_pool:
            # acc[mh][n] = pmi_vec[b half mh, v_out 512-chunk n] — full banks
            accs = [[acc_pool.tile([128, 512], F32, tag=f"acc{mh}{n}",
                                   name=f"acc{mh}{n}") for n in range(NC2)]
                    for mh in range(B // 128)]

            pmi_hi_src = pmi_hi_d.rearrange("(kb kk p) n -> kb p kk n",
                                            p=128, kk=KB)
            pmi_lo_src = (pmi_lo_d.rearrange("(kb kk p) n -> kb p kk n",
                                             p=128, kk=KB)
                          if two_pass else None)
            for kb in range(KT // KB):
                hi = pmi_pool.tile([128, KB, VS], mm_dt, tag="hi", name="hi",
                                   bufs=nbufs)
                lo = None
                if two_pass:
                    lo = pmi_pool.tile([128, KB, VS], mm_dt, tag="lo",
                                       name="lo", bufs=nbufs)
                    nc.sync.dma_start(hi[:], pmi_hi_src[kb])
                    nc.scalar.dma_start(lo[:], pmi_lo_src[kb])
                elif kb % 2 == 0:
                    nc.sync.dma_start(hi[:], pmi_hi_src[kb])
                else:
                    nc.scalar.dma_start(hi[:], pmi_hi_src[kb])
                for kk in range(KB):
                    k = kb * KB + kk
                    for mh in range(B // 128):
                        lhsT = bow_sb[:, k, mh * 128:(mh + 1) * 128]
                        for n in range(NC2):
                            nc.tensor.matmul(
                                accs[mh][n][:], lhsT,
                                hi[:, kk, n * 512:(n + 1) * 512],
                                start=(k == 0),
                                stop=(k == KT - 1 and not two_pass))
                            if two_pass:
                                nc.tensor.matmul(
                                    accs[mh][n][:], lhsT,
                                    lo[:, kk, n * 512:(n + 1) * 512],
                                    start=False, stop=(k == KT - 1))

            # drain pmi_vec [b, v_out] to SBUF (fp32)
            for mh in range(B // 128):
                for n in range(NC2):
                    nc.vector.tensor_copy(
                        pvec_sb[:, mh, n * 512:(n + 1) * 512],
                        accs[mh][n][:])

        # ---------------- transpose pmi_vec -> pmi_vecT [v_out, b] ----------
        pvT_sb = const.tile([128, MB, B], F32, name="pvT_sb")
        with tc.tile_pool(name="ptr", bufs=1, space="PSUM") as ptr_pool:
            for m in range(MB):
                for mh in range(B // 128):
                    pt = ptr_pool.tile([128, 128], F32, tag="pt", name="pt",
                                       bufs=4)
                    nc.tensor.transpose(
                        pt[:], pvec_sb[:, mh, m * 128:(m + 1) * 128],
                        ident[:])
                    nc.vector.tensor_copy(
                        pvT_sb[:, m, mh * 128:(mh + 1) * 128], pt[:])

        if debug_pv:
            nc.gpsimd.dma_start(pv_d.rearrange("(m p) b -> p m b", p=128),
                                pvT_sb[:])

        # ---------------- matmul #2: zpart = pmi_vec_slice @ wlT ------------
        out_pool = ctx.enter_context(tc.tile_pool(name="outp", bufs=2))
        with tc.tile_pool(name="pz2", bufs=1, space="PSUM") as pz2:
            for bh in range(B // 128):
                zp = pz2.tile([128, ZL], F32, tag="zp", name="zp", bufs=2)
                for m in range(MB):
                    nc.tensor.matmul(zp[:],
                                     pvT_sb[:, m, bh * 128:(bh + 1) * 128],
                                     wl_sb[:, m, :],
                                     start=(m == 0), stop=(m == MB - 1))
                zout = out_pool.tile([128, ZL], F32, tag="zout", name="zout")
                nc.vector.tensor_copy(zout[:], zp[:])
                nc.gpsimd.dma_start(zpart_d[bh * 128:(bh + 1) * 128, :],
                                    zout[:])

    nc.compile()
    return nc


_PROGRAM_CACHE = {}


def _get_program(mode, mm2_mode, debug_pv=False):
    key = (mode, mm2_mode, debug_pv)
    if key not in _PROGRAM_CACHE:
        _PROGRAM_CACHE[key] = _build_program(mode, mm2_mode, debug_pv)
    return _PROGRAM_CACHE[key]


def kernel(reports_ids, PMI, W_lang, corr, W1, b1, W2, b2,
           _mode=None, _mm2_mode=None, _trace=False, _trace_out=None,
           _debug_pv=False):
    mode = MODE if _mode is None else _mode
    mm2_mode = MM2_MODE if _mm2_mode is None else _mm2_mode
    _, np_dt = _mode_dtypes(mode)
    mm2_np = np.float32
    two_pass = mode == "bf16x2"

    ids = np.asarray(reports_ids)
    PMI = np.asarray(PMI, dtype=np.float32)
    W_lang = np.asarray(W_lang, dtype=np.float32)
    corr = np.asarray(corr, dtype=np.float32)
    W1 = np.asarray(W1, dtype=np.float32)
    b1 = np.asarray(b1, dtype=np.float32)
    W2 = np.asarray(W2, dtype=np.float32)
    b2 = np.asarray(b2, dtype=np.float32)

    # host: exact integer token counts (the bag-of-words numerator)
    mask = (ids >= 0) & (ids < V)
    idx = np.clip(ids, 0, V - 1).astype(np.int64)
    counts = np.zeros((B, V), np.float32)
    np.add.at(counts, (np.broadcast_to(np.arange(B)[:, None], ids.shape), idx),
              mask.astype(np.float32))
    denom = np.maximum(counts.sum(axis=1), 1.0).astype(np.float32)
    bowT = np.ascontiguousarray(counts.T).astype(np_dt)          # [V, B]

    if two_pass:
        pmi_hi = PMI.astype(np_dt)
        pmi_lo = (PMI - pmi_hi.astype(np.float32)).astype(np_dt)
    else:
        pmi_hi = PMI.astype(np_dt)
        pmi_lo = None

    w1T = np.ascontiguousarray(W1.T)                             # [196, 512]
    corrT = np.ascontiguousarray(corr.reshape(-1)[:, None])      # [196, 1]
    b1c = np.ascontiguousarray(b1.reshape(H // 128, 128).T)      # [128, 4]
    w2T = np.ascontiguousarray(W2.T)                             # [512, 256]
    b2c = np.ascontiguousarray(b2.reshape(ZC // 128, 128).T)     # [128, 2]

    in_maps = []
    for c in range(NCORES):
        sl = slice(c * VS, (c + 1) * VS)
        m = {
            "pmi_hi": np.ascontiguousarray(pmi_hi[:, sl]),
            "bowT": bowT,
            "wlT": np.ascontiguousarray(W_lang[:, sl].T).astype(mm2_np),
            "w1T": w1T, "corrT": corrT, "b1c": b1c,
            "w2T": w2T, "b2c": b2c,
        }
        if two_pass:
            m["pmi_lo"] = np.ascontiguousarray(pmi_lo[:, sl])
        in_maps.append(m)

    nc = _get_program(mode, mm2_mode, _debug_pv)
    res = run_bass_kernel_spmd(nc, in_maps, core_ids=list(range(NCORES)),
                               trace=_trace)
    if _trace_out is not None:
        _trace_out["exec_time_ns"] = res.exec_time_ns
        _trace_out["results"] = res

    z_lang = np.zeros((B, ZL), np.float64)
    for c in range(NCORES):
        z_lang += res.results[c]["zpart"].astype(np.float64)
    z_lang = (z_lang / denom[:, None]).astype(np.float32)
    zc = res.results[0]["zcT"].reshape(ZC)
    out = np.empty((B, ZL + ZC), np.float32)
    out[:, :ZL] = z_lang
    out[:, ZL:] = zc[None, :]
    return out


# revision 13
# speedup vs baseline: 1.0208x; 1.0208x over previous
"""Trainium2 Bass kernel for nn_ConfounderEncoder.

Computation (full shapes):
    bow[b,v]   = normalized bag-of-words histogram of reports_ids   [256, 8192]
    pmi_vec    = bow @ PMI                                          [256, 8192]
    z_lang     = pmi_vec @ W_lang.T                                 [256, 256]
    h          = gelu(corr.flatten() @ W1.T + b1)                   [1, 512]
    z_corr     = broadcast(h @ W2.T + b2)                           [256, 256]
    out        = concat([z_lang, z_corr], -1)                       [256, 512]

Strategy: PMI (256 MB) is the dominant memory traffic; it is column-sharded
across 8 NeuronCores (each core owns a [8192, 1024] slice, read exactly once).
bow is sent as exact integer token counts (transposed, [8192, 256]) — integers
<= 128 are exact in bf16 — and the 1/count normalization is applied to the
final z_lang partials on the host (z_lang is linear in bow rows).

On each core, matmul #1 uses the streamed PMI tile as the *stationary* operand
so the result comes out transposed (pmi_vecT [v_out, b]) — exactly the layout
matmul #2 (contract v_out against W_langT) needs, avoiding any on-chip
transpose. Each core emits a [256, 256] z_lang partial; the host sums the 8
partials (the cross-core reduction commutes through the linear matmul #2).
The tiny corr MLP runs replicated on every core; core 0's result is used.

Precision modes for the big matmul ("bf16x2" default): PMI is split on the
host into hi = bf16(PMI) and lo = bf16(PMI - hi); both accumulate into the
same fp32 PSUM. Error ~1e-5 relative, at 4x the fp32 PE throughput.
"""

import numpy as np
import ml_dtypes
from contextlib import ExitStack

import concourse.bass as bass
import concourse.mybir as mybir
import concourse.tile as tile
from concourse import bacc
from concourse.bass_utils import run_bass_kernel_spmd
from concourse.masks import make_identity

B, T, V, ZL, ZC, L, H = 256, 128, 8192, 256, 256, 14, 512
LL = L * L            # 196
NCORES = 8
VS = V // NCORES      # 1024 pmi columns per core
KT = V // 128         # 64 k tiles over v_in
MB = VS // 128        # 8 v_out blocks per core

MODE = "bf16x2"       # "f32" | "f32r" | "bf16" | "bf16x2"
MM2_MODE = "f32"      # "f32" | "f32r"

F32 = mybir.dt.float32
BF16 = mybir.dt.bfloat16


def _mode_dtypes(mode):
    if mode in ("bf16", "bf16x2"):
        return BF16, ml_dtypes.bfloat16
    if mode == "f32r":
        return mybir.dt.float32r, np.float32
    return F32, np.float32


def _build_program(mode, mm2_mode, debug_pv=False):
    mm_dt, _ = _mode_dtypes(mode)
    mm2_dt = mybir.dt.float32r if mm2_mode == "f32r" else F32
    two_pass = mode == "bf16x2"

    nc = bacc.Bacc("TRN2", target_bir_lowering=False, debug=False,
                   num_devices=NCORES)

    pmi_hi_d = nc.dram_tensor("pmi_hi", [V, VS], mm_dt, kind="ExternalInput")
    pmi_lo_d = (nc.dram_tensor("pmi_lo", [V, VS], mm_dt, kind="ExternalInput")
                if two_pass else None)
    bowT_d = nc.dram_tensor("bowT", [V, B], mm_dt, kind="ExternalInput")
    wlT_d = nc.dram_tensor("wlT", [VS, ZL], mm2_dt, kind="ExternalInput")
    w1T_d = nc.dram_tensor("w1T", [LL, H], F32, kind="ExternalInput")
    corrT_d = nc.dram_tensor("corrT", [LL, 1], F32, kind="ExternalInput")
    b1_d = nc.dram_tensor("b1c", [128, H // 128], F32, kind="ExternalInput")
    w2T_d = nc.dram_tensor("w2T", [H, ZC], F32, kind="ExternalInput")
    b2_d = nc.dram_tensor("b2c", [128, ZC // 128], F32, kind="ExternalInput")

    zpart_d = nc.dram_tensor("zpart", [B, ZL], F32, kind="ExternalOutput")
    zc_d = nc.dram_tensor("zcT", [ZC, 1], F32, kind="ExternalOutput")
    pv_d = (nc.dram_tensor("pvT", [VS, B], F32, kind="ExternalOutput")
            if debug_pv else None)

    with tile.TileContext(nc) as tc, ExitStack() as ctx:
        const = ctx.enter_context(tc.tile_pool(name="const", bufs=1))

        # ---------------- corr MLP (tiny, replicated) ----------------
        with tc.tile_pool(name="psc", bufs=1, space="PSUM") as psc:
            w1_sb = const.tile([128, 2, H], F32, name="w1_sb")
            nc.gpsimd.dma_start(w1_sb[:, 0, :], w1T_d[0:128, :])
            nc.gpsimd.dma_start(w1_sb[0:LL - 128, 1, :], w1T_d[128:LL, :])
            corr_sb = const.tile([128, 2], F32, name="corr_sb")
            nc.gpsimd.dma_start(corr_sb[:, 0:1], corrT_d[0:128, :])
            nc.gpsimd.dma_start(corr_sb[0:LL - 128, 1:2], corrT_d[128:LL, :])
            b1_sb = const.tile([128, H // 128], F32, name="b1_sb")
            nc.gpsimd.dma_start(b1_sb[:], b1_d[:])
            w2_sb = const.tile([128, H // 128, ZC], F32, name="w2_sb")
            nc.gpsimd.dma_start(
                w2_sb[:], w2T_d.rearrange("(k p) z -> p k z", p=128))
            b2_sb = const.tile([128, ZC // 128], F32, name="b2_sb")
            nc.gpsimd.dma_start(b2_sb[:], b2_d[:])

            h_sb = const.tile([128, H // 128], F32, name="h_sb")
            for m in range(H // 128):
                ph = psc.tile([128, 1], F32, tag="ph", name="ph", bufs=2)
                nc.tensor.matmul(ph[:], w1_sb[:, 0, m * 128:(m + 1) * 128],
                                 corr_sb[:, 0:1], start=True, stop=False)
                nc.tensor.matmul(ph[:],
                                 w1_sb[0:LL - 128, 1, m * 128:(m + 1) * 128],
                                 corr_sb[0:LL - 128, 1:2],
                                 start=False, stop=True)
                nc.scalar.activation(h_sb[:, m:m + 1], ph[:],
                                     mybir.ActivationFunctionType.Gelu,
                                     bias=b1_sb[:, m:m + 1], scale=1.0)

            zc_sb = const.tile([128, ZC // 128], F32, name="zc_sb")
            for m in range(ZC // 128):
                pz = psc.tile([128, 1], F32, tag="pz", name="pz", bufs=2)
                for k in range(H // 128):
                    nc.tensor.matmul(pz[:],
                                     w2_sb[:, k, m * 128:(m + 1) * 128],
                                     h_sb[:, k:k + 1],
                                     start=(k == 0), stop=(k == H // 128 - 1))
                nc.scalar.activation(zc_sb[:, m:m + 1], pz[:],
                                     mybir.ActivationFunctionType.Identity,
                                     bias=b2_sb[:, m:m + 1], scale=1.0)
            nc.gpsimd.dma_start(
                zc_d.rearrange("(m p) one -> p (m one)", p=128), zc_sb[:])

        # ---------------- matmul #1: pmi_vec = bow @ PMI_slice --------------
        # bow (transposed, integer counts) is the stationary operand: each
        # [128,128] lhsT tile is reused across 4-8 N=512 matmuls. PMI streams
        # as the moving operand in KB-k-tile batches (big DMAs), split across
        # the two HWDGE rings (sync / scalar).
        KB = 8                      # k-tiles per DMA batch
        NW = 512                    # moving-operand width (HW max out free)
        NC2 = VS // NW              # v_out chunks per core
        bow_sb = const.tile([128, KT, B], mm_dt, name="bow_sb")
        bow_src = bowT_d.rearrange("(k p) b -> p k b", p=128)
        for c in range(4):
            kc = KT // 4
            nc.gpsimd.dma_start(bow_sb[:, c * kc:(c + 1) * kc, :],
                                bow_src[:, c * kc:(c + 1) * kc, :])

        wl_sb = const.tile([128, MB, ZL], mm2_dt, name="wl_sb")
        nc.gpsimd.dma_start(wl_sb[:],
                            wlT_d.rearrange("(k p) z -> p k z", p=128))
        ident = const.tile([128, 128], F32, name="ident")
        make_identity(nc, ident[:])

        pmi_pool = ctx.enter_context(tc.tile_pool(name="pmi", bufs=1))
        pvec_sb = const.tile([128, B // 128, VS], F32, name="pvec_sb")
        nbufs = 3 if mm_dt == BF16 else 2
        with tc.tile_pool(name="acc", bufs=1, space="PSUM") as acc_pool:
            # acc[mh][n] = pmi_vec[b half mh, v_out 512-chunk n] — full banks
            accs = [[acc_pool.tile([128, NW], F32, tag=f"acc{mh}{n}",
                                   name=f"acc{mh}{n}") for n in range(NC2)]
                    for mh in range(B // 128)]

            pmi_hi_src = pmi_hi_d.rearrange("(kb kk p) n -> kb p kk n",
                                            p=128, kk=KB)
            pmi_lo_src = (pmi_lo_d.rearrange("(kb kk p) n -> kb p kk n",
                                             p=128, kk=KB)
                          if two_pass else None)
            for kb in range(KT // KB):
                hi = pmi_pool.tile([128, KB, VS], mm_dt, tag="hi", name="hi",
                                   bufs=nbufs)
                lo = None
                if two_pass:
                    lo = pmi_pool.tile([128, KB, VS], mm_dt, tag="lo",
                                       name="lo", bufs=nbufs)
                    nc.sync.dma_start(hi[:], pmi_hi_src[kb])
                    nc.scalar.dma_start(lo[:], pmi_lo_src[kb])
                elif kb % 2 == 0:
                    nc.sync.dma_start(hi[:], pmi_hi_src[kb])
                else:
                    nc.scalar.dma_start(hi[:], pmi_hi_src[kb])
                for kk in range(KB):
                    k = kb * KB + kk
                    for mh in range(B // 128):
                        lhsT = bow_sb[:, k, mh * 128:(mh + 1) * 128]
                        for n in range(NC2):
                            nc.tensor.matmul(
                                accs[mh][n][:], lhsT,
                                hi[:, kk, n * NW:(n + 1) * NW],
                                start=(k == 0),
                                stop=(k == KT - 1 and not two_pass))
                            if two_pass:
                                nc.tensor.matmul(
                                    accs[mh][n][:], lhsT,
                                    lo[:, kk, n * NW:(n + 1) * NW],
                                    start=False, stop=(k == KT - 1))

            # drain pmi_vec [b, v_out] to SBUF (fp32)
            for mh in range(B // 128):
                for n in range(NC2):
                    nc.vector.tensor_copy(
                        pvec_sb[:, mh, n * NW:(n + 1) * NW],
                        accs[mh][n][:])

        # ---------------- transpose pmi_vec -> pmi_vecT [v_out, b] ----------
        pvT_sb = const.tile([128, MB, B], F32, name="pvT_sb")
        with tc.tile_pool(name="ptr", bufs=1, space="PSUM") as ptr_pool:
            for m in range(MB):
                for mh in range(B // 128):
                    pt = ptr_pool.tile([128, 128], F32, tag="pt", name="pt",
                                       bufs=4)
                    nc.tensor.transpose(
                        pt[:], pvec_sb[:, mh, m * 128:(m + 1) * 128],
                        ident[:])
                    nc.vector.tensor_copy(
                        pvT_sb[:, m, mh * 128:(mh + 1) * 128], pt[:])

        if debug_pv:
            nc.gpsimd.dma_start(pv_d.rearrange("(m p) b -> p m b", p=128),
                                pvT_sb[:])

        # ---------------- matmul #2: zpart = pmi_vec_slice @ wlT ------------
        out_pool = ctx.enter_context(tc.tile_pool(name="outp", bufs=2))
        with tc.tile_pool(name="pz2", bufs=1, space="PSUM") as pz2:
            for bh in range(B // 128):
                zp = pz2.tile([128, ZL], F32, tag="zp", name="zp", bufs=2)
                for m in range(MB):
                    nc.tensor.matmul(zp[:],
                                     pvT_sb[:, m, bh * 128:(bh + 1) * 128],
                                     wl_sb[:, m, :],
                                     start=(m == 0), stop=(m == MB - 1))
                zout = out_pool.tile([128, ZL], F32, tag="zout", name="zout")
                nc.vector.tensor_copy(zout[:], zp[:])
                nc.gpsimd.dma_start(zpart_d[bh * 128:(bh + 1) * 128, :],
                                    zout[:])

    nc.compile()
    return nc


_PROGRAM_CACHE = {}


def _get_program(mode, mm2_mode, debug_pv=False):
    key = (mode, mm2_mode, debug_pv)
    if key not in _PROGRAM_CACHE:
        _PROGRAM_CACHE[key] = _build_program(mode, mm2_mode, debug_pv)
    return _PROGRAM_CACHE[key]


def kernel(reports_ids, PMI, W_lang, corr, W1, b1, W2, b2,
           _mode=None, _mm2_mode=None, _trace=False, _trace_out=None,
           _debug_pv=False):
    mode = MODE if _mode is None else _mode
    mm2_mode = MM2_MODE if _mm2_mode is None else _mm2_mode
    _, np_dt = _mode_dtypes(mode)
    mm2_np = np.float32
    two_pass = mode == "bf16x2"

    ids = np.asarray(reports_ids)
    PMI = np.asarray(PMI, dtype=np.float32)
    W_lang = np.asarray(W_lang, dtype=np.float32)
    corr = np.asarray(corr, dtype=np.float32)
    W1 = np.asarray(W1, dtype=np.float32)
    b1 = np.asarray(b1, dtype=np.float32)
    W2 = np.asarray(W2, dtype=np.float32)
    b2 = np.asarray(b2, dtype=np.float32)

    # host: exact integer token counts (the bag-of-words numerator)
    mask = (ids >= 0) & (ids < V)
    idx = np.clip(ids, 0, V - 1).astype(np.int64)
    counts = np.zeros((B, V), np.float32)
    np.add.at(counts, (np.broadcast_to(np.arange(B)[:, None], ids.shape), idx),
              mask.astype(np.float32))
    denom = np.maximum(counts.sum(axis=1), 1.0).astype(np.float32)
    bowT = np.ascontiguousarray(counts.T).astype(np_dt)          # [V, B]

    if two_pass:
        pmi_hi = PMI.astype(np_dt)
        pmi_lo = (PMI - pmi_hi.astype(np.float32)).astype(np_dt)
    else:
        pmi_hi = PMI.astype(np_dt)
        pmi_lo = None

    w1T = np.ascontiguousarray(W1.T)                             # [196, 512]
    corrT = np.ascontiguousarray(corr.reshape(-1)[:, None])      # [196, 1]
    b1c = np.ascontiguousarray(b1.reshape(H // 128, 128).T)      # [128, 4]
    w2T = np.ascontiguousarray(W2.T)                             # [512, 256]
    b2c = np.ascontiguousarray(b2.reshape(ZC // 128, 128).T)     # [128, 2]

    in_maps = []
    for c in range(NCORES):
        sl = slice(c * VS, (c + 1) * VS)
        m = {
            "pmi_hi": np.ascontiguousarray(pmi_hi[:, sl]),
            "bowT": bowT,
            "wlT": np.ascontiguousarray(W_lang[:, sl].T).astype(mm2_np),
            "w1T": w1T, "corrT": corrT, "b1c": b1c,
            "w2T": w2T, "b2c": b2c,
        }
        if two_pass:
            m["pmi_lo"] = np.ascontiguousarray(pmi_lo[:, sl])
        in_maps.append(m)

    nc = _get_program(mode, mm2_mode, _debug_pv)
    res = run_bass_kernel_spmd(nc, in_maps, core_ids=list(range(NCORES)),
                               trace=_trace)
    if _trace_out is not None:
        _trace_out["exec_time_ns"] = res.exec_time_ns
        _trace_out["results"] = res

    z_lang = np.zeros((B, ZL), np.float64)
    for c in range(NCORES):
        z_lang += res.results[c]["zpart"].astype(np.float64)
    z_lang = (z_lang / denom[:, None]).astype(np.float32)
    zc = res.results[0]["zcT"].reshape(ZC)
    out = np.empty((B, ZL + ZC), np.float32)
    out[:, :ZL] = z_lang
    out[:, ZL:] = zc[None, :]
    return out


# revision 19
# speedup vs baseline: 1.5521x; 1.5205x over previous
"""Trainium2 Bass kernel for nn_ConfounderEncoder.

Computation (full shapes):
    bow[b,v]   = normalized bag-of-words histogram of reports_ids   [256, 8192]
    pmi_vec    = bow @ PMI                                          [256, 8192]
    z_lang     = pmi_vec @ W_lang.T                                 [256, 256]
    h          = gelu(corr.flatten() @ W1.T + b1)                   [1, 512]
    z_corr     = broadcast(h @ W2.T + b2)                           [256, 256]
    out        = concat([z_lang, z_corr], -1)                       [256, 512]

Strategy: PMI (256 MB) is the dominant memory traffic; it is column-sharded
across 8 NeuronCores (each core owns a [8192, 1024] slice, read exactly once).
bow is sent as exact integer token counts (transposed, [8192, 256]) — integers
<= 128 are exact in bf16 — and the 1/count normalization is applied to the
final z_lang partials on the host (z_lang is linear in bow rows).

On each core, matmul #1 uses the streamed PMI tile as the *stationary* operand
so the result comes out transposed (pmi_vecT [v_out, b]) — exactly the layout
matmul #2 (contract v_out against W_langT) needs, avoiding any on-chip
transpose. Each core emits a [256, 256] z_lang partial; the host sums the 8
partials (the cross-core reduction commutes through the linear matmul #2).
The tiny corr MLP runs replicated on every core; core 0's result is used.

Precision modes for the big matmul ("bf16x2" default): PMI is split on the
host into hi = bf16(PMI) and lo = bf16(PMI - hi); both accumulate into the
same fp32 PSUM. Error ~1e-5 relative, at 4x the fp32 PE throughput.
"""

import numpy as np
import ml_dtypes
from contextlib import ExitStack

import concourse.bass as bass
import concourse.mybir as mybir
import concourse.tile as tile
from concourse import bacc
from concourse.bass_utils import run_bass_kernel_spmd
from concourse.masks import make_identity

B, T, V, ZL, ZC, L, H = 256, 128, 8192, 256, 256, 14, 512
LL = L * L            # 196
NCORES = 8
VS = V // NCORES      # 1024 pmi columns per core
KT = V // 128         # 64 k tiles over v_in
MB = VS // 128        # 8 v_out blocks per core

MODE = "bf16x2"       # "f32" | "f32r" | "bf16" | "bf16x2"
MM2_MODE = "f32"      # "f32" | "f32r"

F32 = mybir.dt.float32
BF16 = mybir.dt.bfloat16


def _mode_dtypes(mode):
    if mode in ("bf16", "bf16x2"):
        return BF16, ml_dtypes.bfloat16
    if mode == "f32r":
        return mybir.dt.float32r, np.float32
    return F32, np.float32


def _build_program(mode, mm2_mode, debug_pv=False):
    mm_dt, _ = _mode_dtypes(mode)
    if mm2_mode == "f32r":
        mm2_dt = mybir.dt.float32r
    elif mm2_mode == "bf16":
        mm2_dt = BF16
    else:
        mm2_dt = F32
    two_pass = mode == "bf16x2"

    nc = bacc.Bacc("TRN2", target_bir_lowering=False, debug=False,
                   num_devices=NCORES)

    pmi_hi_d = nc.dram_tensor("pmi_hi", [V, VS], mm_dt, kind="ExternalInput")
    pmi_lo_d = (nc.dram_tensor("pmi_lo", [V, VS], mm_dt, kind="ExternalInput")
                if two_pass else None)
    bowT_d = nc.dram_tensor("bowT", [V, B], mm_dt, kind="ExternalInput")
    wlT_d = nc.dram_tensor("wlT", [VS, ZL], mm2_dt, kind="ExternalInput")
    w1T_d = nc.dram_tensor("w1T", [LL, H], F32, kind="ExternalInput")
    corrT_d = nc.dram_tensor("corrT", [LL, 1], F32, kind="ExternalInput")
    b1_d = nc.dram_tensor("b1c", [128, H // 128], F32, kind="ExternalInput")
    w2T_d = nc.dram_tensor("w2T", [H, ZC], F32, kind="ExternalInput")
    b2_d = nc.dram_tensor("b2c", [128, ZC // 128], F32, kind="ExternalInput")

    zpart_d = nc.dram_tensor("zpart", [B, ZL], F32, kind="ExternalOutput")
    zc_d = nc.dram_tensor("zcT", [ZC, 1], F32, kind="ExternalOutput")
    pv_d = (nc.dram_tensor("pvT", [VS, B], F32, kind="ExternalOutput")
            if debug_pv else None)

    with tile.TileContext(nc) as tc, ExitStack() as ctx:
        const = ctx.enter_context(tc.tile_pool(name="const", bufs=1))

        # ---------------- corr MLP (tiny, replicated) ----------------
        with tc.tile_pool(name="psc", bufs=1, space="PSUM") as psc:
            w1_sb = const.tile([128, 2, H], F32, name="w1_sb")
            nc.gpsimd.dma_start(w1_sb[:, 0, :], w1T_d[0:128, :])
            nc.gpsimd.dma_start(w1_sb[0:LL - 128, 1, :], w1T_d[128:LL, :])
            corr_sb = const.tile([128, 2], F32, name="corr_sb")
            nc.gpsimd.dma_start(corr_sb[:, 0:1], corrT_d[0:128, :])
            nc.gpsimd.dma_start(corr_sb[0:LL - 128, 1:2], corrT_d[128:LL, :])
            b1_sb = const.tile([128, H // 128], F32, name="b1_sb")
            nc.gpsimd.dma_start(b1_sb[:], b1_d[:])
            w2_sb = const.tile([128, H // 128, ZC], F32, name="w2_sb")
            nc.gpsimd.dma_start(
                w2_sb[:], w2T_d.rearrange("(k p) z -> p k z", p=128))
            b2_sb = const.tile([128, ZC // 128], F32, name="b2_sb")
            nc.gpsimd.dma_start(b2_sb[:], b2_d[:])

            h_sb = const.tile([128, H // 128], F32, name="h_sb")
            for m in range(H // 128):
                ph = psc.tile([128, 1], F32, tag="ph", name="ph", bufs=2)
                nc.tensor.matmul(ph[:], w1_sb[:, 0, m * 128:(m + 1) * 128],
                                 corr_sb[:, 0:1], start=True, stop=False)
                nc.tensor.matmul(ph[:],
                                 w1_sb[0:LL - 128, 1, m * 128:(m + 1) * 128],
                                 corr_sb[0:LL - 128, 1:2],
                                 start=False, stop=True)
                nc.scalar.activation(h_sb[:, m:m + 1], ph[:],
                                     mybir.ActivationFunctionType.Gelu,
                                     bias=b1_sb[:, m:m + 1], scale=1.0)

            zc_sb = const.tile([128, ZC // 128], F32, name="zc_sb")
            for m in range(ZC // 128):
                pz = psc.tile([128, 1], F32, tag="pz", name="pz", bufs=2)
                for k in range(H // 128):
                    nc.tensor.matmul(pz[:],
                                     w2_sb[:, k, m * 128:(m + 1) * 128],
                                     h_sb[:, k:k + 1],
                                     start=(k == 0), stop=(k == H // 128 - 1))
                nc.scalar.activation(zc_sb[:, m:m + 1], pz[:],
                                     mybir.ActivationFunctionType.Identity,
                                     bias=b2_sb[:, m:m + 1], scale=1.0)
            nc.gpsimd.dma_start(
                zc_d.rearrange("(m p) one -> p (m one)", p=128), zc_sb[:])

        # ---------------- matmul #1: pmi_vec = bow @ PMI_slice --------------
        # bow (transposed, integer counts) is the stationary operand: each
        # [128,128] lhsT tile is reused across 4-8 N=512 matmuls. PMI streams
        # as the moving operand in KB-k-tile batches (big DMAs), split across
        # the two HWDGE rings (sync / scalar).
        KB = 8                      # k-tiles per DMA batch
        NW = 512                    # moving-operand width (HW max out free)
        NC2 = VS // NW              # v_out chunks per core
        bow_sb = const.tile([128, KT, B], mm_dt, name="bow_sb")
        bow_src = bowT_d.rearrange("(k p) b -> p k b", p=128)
        kc = KT // 4
        # first chunks on the fast HWDGE rings so matmuls can start early
        nc.sync.dma_start(bow_sb[:, 0:kc, :], bow_src[:, 0:kc, :])
        nc.scalar.dma_start(bow_sb[:, kc:2 * kc, :], bow_src[:, kc:2 * kc, :])
        for c in (2, 3):
            nc.gpsimd.dma_start(bow_sb[:, c * kc:(c + 1) * kc, :],
                                bow_src[:, c * kc:(c + 1) * kc, :])

        wl_sb = const.tile([128, MB, ZL], mm2_dt, name="wl_sb")
        nc.gpsimd.dma_start(wl_sb[:],
                            wlT_d.rearrange("(k p) z -> p k z", p=128))
        ident = const.tile([128, 128], F32, name="ident")
        make_identity(nc, ident[:])

        pmi_pool = ctx.enter_context(tc.tile_pool(name="pmi", bufs=1))
        pvec_sb = const.tile([128, B // 128, VS], F32, name="pvec_sb")
        if mm_dt != BF16:
            nbufs = 2
        elif two_pass:
            nbufs = 3
        else:
            nbufs = 4
        with tc.tile_pool(name="acc", bufs=1, space="PSUM") as acc_pool:
            # acc[mh][n] = pmi_vec[b half mh, v_out 512-chunk n] — full banks
            accs = [[acc_pool.tile([128, NW], F32, tag=f"acc{mh}{n}",
                                   name=f"acc{mh}{n}") for n in range(NC2)]
                    for mh in range(B // 128)]

            pmi_hi_src = pmi_hi_d.rearrange("(kb kk p) n -> kb p kk n",
                                            p=128, kk=KB)
            pmi_lo_src = (pmi_lo_d.rearrange("(kb kk p) n -> kb p kk n",
                                             p=128, kk=KB)
                          if two_pass else None)
            for kb in range(KT // KB):
                hi = pmi_pool.tile([128, KB, VS], mm_dt, tag="hi", name="hi",
                                   bufs=nbufs)
                lo = None
                if two_pass:
                    lo = pmi_pool.tile([128, KB, VS], mm_dt, tag="lo",
                                       name="lo", bufs=nbufs)
                    if kb == 0:
                        h2 = KB // 2
                        nc.sync.dma_start(hi[:, 0:h2, :],
                                          pmi_hi_src[kb][:, 0:h2, :])
                        nc.scalar.dma_start(hi[:, h2:KB, :],
                                            pmi_hi_src[kb][:, h2:KB, :])
                        nc.sync.dma_start(lo[:, 0:h2, :],
                                          pmi_lo_src[kb][:, 0:h2, :])
                        nc.scalar.dma_start(lo[:, h2:KB, :],
                                            pmi_lo_src[kb][:, h2:KB, :])
                    else:
                        nc.sync.dma_start(hi[:], pmi_hi_src[kb])
                        nc.scalar.dma_start(lo[:], pmi_lo_src[kb])
                elif kb == 0:
                    h2 = KB // 2
                    nc.sync.dma_start(hi[:, 0:h2, :],
                                      pmi_hi_src[kb][:, 0:h2, :])
                    nc.scalar.dma_start(hi[:, h2:KB, :],
                                        pmi_hi_src[kb][:, h2:KB, :])
                elif kb % 2 == 0:
                    nc.sync.dma_start(hi[:], pmi_hi_src[kb])
                else:
                    nc.scalar.dma_start(hi[:], pmi_hi_src[kb])
                for kk in range(KB):
                    k = kb * KB + kk
                    for mh in range(B // 128):
                        lhsT = bow_sb[:, k, mh * 128:(mh + 1) * 128]
                        for n in range(NC2):
                            nc.tensor.matmul(
                                accs[mh][n][:], lhsT,
                                hi[:, kk, n * NW:(n + 1) * NW],
                                start=(k == 0),
                                stop=(k == KT - 1 and not two_pass))
                            if two_pass:
                                nc.tensor.matmul(
                                    accs[mh][n][:], lhsT,
                                    lo[:, kk, n * NW:(n + 1) * NW],
                                    start=False, stop=(k == KT - 1))

            # drain pmi_vec [b, v_out] to SBUF (fp32)
            for mh in range(B // 128):
                for n in range(NC2):
                    nc.vector.tensor_copy(
                        pvec_sb[:, mh, n * NW:(n + 1) * NW],
                        accs[mh][n][:])

        # ---------------- transpose pmi_vec -> pmi_vecT [v_out, b] ----------
        pvT_sb = const.tile([128, MB, B], mm2_dt, name="pvT_sb")
        with tc.tile_pool(name="ptr", bufs=1, space="PSUM") as ptr_pool:
            for m in range(MB):
                for mh in range(B // 128):
                    pt = ptr_pool.tile([128, 128], F32, tag="pt", name="pt",
                                       bufs=4)
                    nc.tensor.transpose(
                        pt[:], pvec_sb[:, mh, m * 128:(m + 1) * 128],
                        ident[:])
                    nc.vector.tensor_copy(
                        pvT_sb[:, m, mh * 128:(mh + 1) * 128], pt[:])

        if debug_pv:
            nc.gpsimd.dma_start(pv_d.rearrange("(m p) b -> p m b", p=128),
                                pvT_sb[:])

        # ---------------- matmul #2: zpart = pmi_vec_slice @ wlT ------------
        out_pool = ctx.enter_context(tc.tile_pool(name="outp", bufs=2))
        with tc.tile_pool(name="pz2", bufs=1, space="PSUM") as pz2:
            for bh in range(B // 128):
                zp = pz2.tile([128, ZL], F32, tag="zp", name="zp", bufs=2)
                for m in range(MB):
                    nc.tensor.matmul(zp[:],
                                     pvT_sb[:, m, bh * 128:(bh + 1) * 128],
                                     wl_sb[:, m, :],
                                     start=(m == 0), stop=(m == MB - 1))
                zout = out_pool.tile([128, ZL], F32, tag="zout", name="zout")
                nc.vector.tensor_copy(zout[:], zp[:])
                nc.gpsimd.dma_start(zpart_d[bh * 128:(bh + 1) * 128, :],
                                    zout[:])

    nc.compile()
    return nc


_PROGRAM_CACHE = {}


def _get_program(mode, mm2_mode, debug_pv=False):
    key = (mode, mm2_mode, debug_pv)
    if key not in _PROGRAM_CACHE:
        _PROGRAM_CACHE[key] = _build_program(mode, mm2_mode, debug_pv)
    return _PROGRAM_CACHE[key]


def kernel(reports_ids, PMI, W_lang, corr, W1, b1, W2, b2,
           _mode=None, _mm2_mode=None, _trace=False, _trace_out=None,
           _debug_pv=False):
    mode = MODE if _mode is None else _mode
    mm2_mode = MM2_MODE if _mm2_mode is None else _mm2_mode
    _, np_dt = _mode_dtypes(mode)
    mm2_np = ml_dtypes.bfloat16 if mm2_mode == "bf16" else np.float32
    two_pass = mode == "bf16x2"

    ids = np.asarray(reports_ids)
    PMI = np.asarray(PMI, dtype=np.float32)
    W_lang = np.asarray(W_lang, dtype=np.float32)
    corr = np.asarray(corr, dtype=np.float32)
    W1 = np.asarray(W1, dtype=np.float32)
    b1 = np.asarray(b1, dtype=np.float32)
    W2 = np.asarray(W2, dtype=np.float32)
    b2 = np.asarray(b2, dtype=np.float32)

    # host: exact integer token counts (the bag-of-words numerator)
    mask = (ids >= 0) & (ids < V)
    idx = np.clip(ids, 0, V - 1).astype(np.int64)
    counts = np.zeros((B, V), np.float32)
    np.add.at(counts, (np.broadcast_to(np.arange(B)[:, None], ids.shape), idx),
              mask.astype(np.float32))
    denom = np.maximum(counts.sum(axis=1), 1.0).astype(np.float32)
    bowT = np.ascontiguousarray(counts.T).astype(np_dt)          # [V, B]

    if two_pass:
        pmi_hi = PMI.astype(np_dt)
        pmi_lo = (PMI - pmi_hi.astype(np.float32)).astype(np_dt)
    else:
        pmi_hi = PMI.astype(np_dt)
        pmi_lo = None

    w1T = np.ascontiguousarray(W1.T)                             # [196, 512]
    corrT = np.ascontiguousarray(corr.reshape(-1)[:, None])      # [196, 1]
    b1c = np.ascontiguousarray(b1.reshape(H // 128, 128).T)      # [128, 4]
    w2T = np.ascontiguousarray(W2.T)                             # [512, 256]
    b2c = np.ascontiguousarray(b2.reshape(ZC // 128, 128).T)     # [128, 2]

    in_maps = []
    for c in range(NCORES):
        sl = slice(c * VS, (c + 1) * VS)
        m = {
            "pmi_hi": np.ascontiguousarray(pmi_hi[:, sl]),
            "bowT": bowT,
            "wlT": np.ascontiguousarray(W_lang[:, sl].T).astype(mm2_np),
            "w1T": w1T, "corrT": corrT, "b1c": b1c,
            "w2T": w2T, "b2c": b2c,
        }
        if two_pass:
            m["pmi_lo"] = np.ascontiguousarray(pmi_lo[:, sl])
        in_maps.append(m)

    nc = _get_program(mode, mm2_mode, _debug_pv)
    res = run_bass_kernel_spmd(nc, in_maps, core_ids=list(range(NCORES)),
                               trace=_trace)
    if _trace_out is not None:
        _trace_out["exec_time_ns"] = res.exec_time_ns
        _trace_out["results"] = res

    z_lang = np.zeros((B, ZL), np.float64)
    for c in range(NCORES):
        z_lang += res.results[c]["zpart"].astype(np.float64)
    z_lang = (z_lang / denom[:, None]).astype(np.float32)
    zc = res.results[0]["zcT"].reshape(ZC)
    out = np.empty((B, ZL + ZC), np.float32)
    out[:, :ZL] = z_lang
    out[:, ZL:] = zc[None, :]
    return out


# revision 21
# speedup vs baseline: 1.6626x; 1.0712x over previous
"""Trainium2 Bass kernel for nn_ConfounderEncoder.

Computation (full shapes):
    bow[b,v]   = normalized bag-of-words histogram of reports_ids   [256, 8192]
    pmi_vec    = bow @ PMI                                          [256, 8192]
    z_lang     = pmi_vec @ W_lang.T                                 [256, 256]
    h          = gelu(corr.flatten() @ W1.T + b1)                   [1, 512]
    z_corr     = broadcast(h @ W2.T + b2)                           [256, 256]
    out        = concat([z_lang, z_corr], -1)                       [256, 512]

Strategy: PMI (256 MB) is the dominant memory traffic; it is column-sharded
across 8 NeuronCores (each core owns a [8192, 1024] slice, read exactly once).
bow is sent as exact integer token counts (transposed, [8192, 256]) — integers
<= 128 are exact in bf16 — and the 1/count normalization is applied to the
final z_lang partials on the host (z_lang is linear in bow rows).

On each core, matmul #1 uses the streamed PMI tile as the *stationary* operand
so the result comes out transposed (pmi_vecT [v_out, b]) — exactly the layout
matmul #2 (contract v_out against W_langT) needs, avoiding any on-chip
transpose. Each core emits a [256, 256] z_lang partial; the host sums the 8
partials (the cross-core reduction commutes through the linear matmul #2).
The tiny corr MLP runs replicated on every core; core 0's result is used.

Precision modes for the big matmul ("bf16x2" default): PMI is split on the
host into hi = bf16(PMI) and lo = bf16(PMI - hi); both accumulate into the
same fp32 PSUM. Error ~1e-5 relative, at 4x the fp32 PE throughput.
"""

import numpy as np
import ml_dtypes
from contextlib import ExitStack

import concourse.bass as bass
import concourse.mybir as mybir
import concourse.tile as tile
from concourse import bacc
from concourse.bass_utils import run_bass_kernel_spmd
from concourse.masks import make_identity

B, T, V, ZL, ZC, L, H = 256, 128, 8192, 256, 256, 14, 512
LL = L * L            # 196
NCORES = 8
VS = V // NCORES      # 1024 pmi columns per core
KT = V // 128         # 64 k tiles over v_in
MB = VS // 128        # 8 v_out blocks per core

MODE = "bf16x2"       # "f32" | "f32r" | "bf16" | "bf16x2"
MM2_MODE = "f32"      # "f32" | "f32r"

F32 = mybir.dt.float32
BF16 = mybir.dt.bfloat16


def _mode_dtypes(mode):
    if mode in ("bf16", "bf16x2"):
        return BF16, ml_dtypes.bfloat16
    if mode == "f32r":
        return mybir.dt.float32r, np.float32
    return F32, np.float32


def _build_program(mode, mm2_mode, debug_pv=False):
    mm_dt, _ = _mode_dtypes(mode)
    if mm2_mode == "f32r":
        mm2_dt = mybir.dt.float32r
    elif mm2_mode == "bf16":
        mm2_dt = BF16
    else:
        mm2_dt = F32
    two_pass = mode == "bf16x2"

    nc = bacc.Bacc("TRN2", target_bir_lowering=False, debug=False,
                   num_devices=NCORES)

    pmi_hi_d = nc.dram_tensor("pmi_hi", [V, VS], mm_dt, kind="ExternalInput")
    pmi_lo_d = (nc.dram_tensor("pmi_lo", [V, VS], mm_dt, kind="ExternalInput")
                if two_pass else None)
    bowT_d = nc.dram_tensor("bowT", [V, B], mm_dt, kind="ExternalInput")
    wlT_d = nc.dram_tensor("wlT", [VS, ZL], mm2_dt, kind="ExternalInput")
    w1T_d = nc.dram_tensor("w1T", [LL, H], F32, kind="ExternalInput")
    corrT_d = nc.dram_tensor("corrT", [LL, 1], F32, kind="ExternalInput")
    b1_d = nc.dram_tensor("b1c", [128, H // 128], F32, kind="ExternalInput")
    w2T_d = nc.dram_tensor("w2T", [H, ZC], F32, kind="ExternalInput")
    b2_d = nc.dram_tensor("b2c", [128, ZC // 128], F32, kind="ExternalInput")

    zpart_d = nc.dram_tensor("zpart", [B, ZL], F32, kind="ExternalOutput")
    zc_d = nc.dram_tensor("zcT", [ZC, 1], F32, kind="ExternalOutput")
    pv_d = (nc.dram_tensor("pvT", [VS, B], F32, kind="ExternalOutput")
            if debug_pv else None)

    with tile.TileContext(nc) as tc, ExitStack() as ctx:
        const = ctx.enter_context(tc.tile_pool(name="const", bufs=1))

        # ---------------- corr MLP (tiny, replicated) ----------------
        with tc.tile_pool(name="psc", bufs=1, space="PSUM") as psc:
            w1_sb = const.tile([128, 2, H], F32, name="w1_sb")
            nc.gpsimd.dma_start(w1_sb[:, 0, :], w1T_d[0:128, :])
            nc.gpsimd.dma_start(w1_sb[0:LL - 128, 1, :], w1T_d[128:LL, :])
            corr_sb = const.tile([128, 2], F32, name="corr_sb")
            nc.gpsimd.dma_start(corr_sb[:, 0:1], corrT_d[0:128, :])
            nc.gpsimd.dma_start(corr_sb[0:LL - 128, 1:2], corrT_d[128:LL, :])
            b1_sb = const.tile([128, H // 128], F32, name="b1_sb")
            nc.gpsimd.dma_start(b1_sb[:], b1_d[:])
            w2_sb = const.tile([128, H // 128, ZC], F32, name="w2_sb")
            nc.gpsimd.dma_start(
                w2_sb[:], w2T_d.rearrange("(k p) z -> p k z", p=128))
            b2_sb = const.tile([128, ZC // 128], F32, name="b2_sb")
            nc.gpsimd.dma_start(b2_sb[:], b2_d[:])

            h_sb = const.tile([128, H // 128], F32, name="h_sb")
            for m in range(H // 128):
                ph = psc.tile([128, 1], F32, tag="ph", name="ph", bufs=2)
                nc.tensor.matmul(ph[:], w1_sb[:, 0, m * 128:(m + 1) * 128],
                                 corr_sb[:, 0:1], start=True, stop=False)
                nc.tensor.matmul(ph[:],
                                 w1_sb[0:LL - 128, 1, m * 128:(m + 1) * 128],
                                 corr_sb[0:LL - 128, 1:2],
                                 start=False, stop=True)
                nc.scalar.activation(h_sb[:, m:m + 1], ph[:],
                                     mybir.ActivationFunctionType.Gelu,
                                     bias=b1_sb[:, m:m + 1], scale=1.0)

            zc_sb = const.tile([128, ZC // 128], F32, name="zc_sb")
            for m in range(ZC // 128):
                pz = psc.tile([128, 1], F32, tag="pz", name="pz", bufs=2)
                for k in range(H // 128):
                    nc.tensor.matmul(pz[:],
                                     w2_sb[:, k, m * 128:(m + 1) * 128],
                                     h_sb[:, k:k + 1],
                                     start=(k == 0), stop=(k == H // 128 - 1))
                nc.scalar.activation(zc_sb[:, m:m + 1], pz[:],
                                     mybir.ActivationFunctionType.Identity,
                                     bias=b2_sb[:, m:m + 1], scale=1.0)
            nc.gpsimd.dma_start(
                zc_d.rearrange("(m p) one -> p (m one)", p=128), zc_sb[:])

        # ---------------- matmul #1: pmi_vec = bow @ PMI_slice --------------
        # bow (transposed, integer counts) is the stationary operand: each
        # [128,128] lhsT tile is reused across 4-8 N=512 matmuls. PMI streams
        # as the moving operand in KB-k-tile batches (big DMAs), split across
        # the two HWDGE rings (sync / scalar).
        KB = 8                      # k-tiles per DMA batch
        NW = 512                    # moving-operand width (HW max out free)
        NC2 = VS // NW              # v_out chunks per core
        bow_sb = const.tile([128, KT, B], mm_dt, name="bow_sb")
        bow_src = bowT_d.rearrange("(k p) b -> p k b", p=128)
        # small first chunks on the fast HWDGE rings so matmuls start early
        nc.sync.dma_start(bow_sb[:, 0:4, :], bow_src[:, 0:4, :])
        nc.scalar.dma_start(bow_sb[:, 4:16, :], bow_src[:, 4:16, :])
        nc.gpsimd.dma_start(bow_sb[:, 16:40, :], bow_src[:, 16:40, :])
        nc.gpsimd.dma_start(bow_sb[:, 40:KT, :], bow_src[:, 40:KT, :])

        wl_sb = const.tile([128, MB, ZL], mm2_dt, name="wl_sb")
        nc.gpsimd.dma_start(wl_sb[:],
                            wlT_d.rearrange("(k p) z -> p k z", p=128))
        ident = const.tile([128, 128], F32, name="ident")
        make_identity(nc, ident[:])

        pmi_pool = ctx.enter_context(tc.tile_pool(name="pmi", bufs=1))
        pvec_sb = const.tile([128, B // 128, VS], F32, name="pvec_sb")
        if mm_dt != BF16:
            nbufs = 2
        elif two_pass:
            nbufs = 3
        else:
            nbufs = 4
        with tc.tile_pool(name="acc", bufs=1, space="PSUM") as acc_pool:
            # acc[mh][n] = pmi_vec[b half mh, v_out 512-chunk n] — full banks
            accs = [[acc_pool.tile([128, NW], F32, tag=f"acc{mh}{n}",
                                   name=f"acc{mh}{n}") for n in range(NC2)]
                    for mh in range(B // 128)]

            pmi_hi_src = pmi_hi_d.rearrange("(kb kk p) n -> kb p kk n",
                                            p=128, kk=KB)
            pmi_lo_src = (pmi_lo_d.rearrange("(kb kk p) n -> kb p kk n",
                                             p=128, kk=KB)
                          if two_pass else None)
            for kb in range(KT // KB):
                hi = pmi_pool.tile([128, KB, VS], mm_dt, tag="hi", name="hi",
                                   bufs=nbufs)
                lo = None
                # every batch split across both HWDGE rings; the first one
                # in quarters so the PE can start as soon as possible
                pieces = ((0, 2), (2, 4), (4, 6), (6, 8)) if kb == 0 \
                    else ((0, 4), (4, 8))
                engs = (nc.sync, nc.scalar)
                for pi, (a, b) in enumerate(pieces):
                    engs[pi % 2].dma_start(hi[:, a:b, :],
                                           pmi_hi_src[kb][:, a:b, :])
                if two_pass:
                    lo = pmi_pool.tile([128, KB, VS], mm_dt, tag="lo",
                                       name="lo", bufs=nbufs)
                    for pi, (a, b) in enumerate(pieces):
                        engs[(pi + 1) % 2].dma_start(lo[:, a:b, :],
                                                     pmi_lo_src[kb][:, a:b, :])
                for kk in range(KB):
                    k = kb * KB + kk
                    for mh in range(B // 128):
                        lhsT = bow_sb[:, k, mh * 128:(mh + 1) * 128]
                        for n in range(NC2):
                            nc.tensor.matmul(
                                accs[mh][n][:], lhsT,
                                hi[:, kk, n * NW:(n + 1) * NW],
                                start=(k == 0),
                                stop=(k == KT - 1 and not two_pass))
                            if two_pass:
                                nc.tensor.matmul(
                                    accs[mh][n][:], lhsT,
                                    lo[:, kk, n * NW:(n + 1) * NW],
                                    start=False, stop=(k == KT - 1))

            # drain pmi_vec [b, v_out] to SBUF (fp32)
            for mh in range(B // 128):
                for n in range(NC2):
                    nc.vector.tensor_copy(
                        pvec_sb[:, mh, n * NW:(n + 1) * NW],
                        accs[mh][n][:])

        # ---------------- transpose pmi_vec -> pmi_vecT [v_out, b] ----------
        pvT_sb = const.tile([128, MB, B], mm2_dt, name="pvT_sb")
        with tc.tile_pool(name="ptr", bufs=1, space="PSUM") as ptr_pool:
            for m in range(MB):
                for mh in range(B // 128):
                    pt = ptr_pool.tile([128, 128], F32, tag="pt", name="pt",
                                       bufs=4)
                    nc.tensor.transpose(
                        pt[:], pvec_sb[:, mh, m * 128:(m + 1) * 128],
                        ident[:])
                    nc.vector.tensor_copy(
                        pvT_sb[:, m, mh * 128:(mh + 1) * 128], pt[:])

        if debug_pv:
            nc.gpsimd.dma_start(pv_d.rearrange("(m p) b -> p m b", p=128),
                                pvT_sb[:])

        # ---------------- matmul #2: zpart = pmi_vec_slice @ wlT ------------
        out_pool = ctx.enter_context(tc.tile_pool(name="outp", bufs=2))
        with tc.tile_pool(name="pz2", bufs=1, space="PSUM") as pz2:
            for bh in range(B // 128):
                zp = pz2.tile([128, ZL], F32, tag="zp", name="zp", bufs=2)
                for m in range(MB):
                    nc.tensor.matmul(zp[:],
                                     pvT_sb[:, m, bh * 128:(bh + 1) * 128],
                                     wl_sb[:, m, :],
                                     start=(m == 0), stop=(m == MB - 1))
                zout = out_pool.tile([128, ZL], F32, tag="zout", name="zout")
                nc.vector.tensor_copy(zout[:], zp[:])
                nc.gpsimd.dma_start(zpart_d[bh * 128:(bh + 1) * 128, :],
                                    zout[:])

    nc.compile()
    return nc


_PROGRAM_CACHE = {}


def _get_program(mode, mm2_mode, debug_pv=False):
    key = (mode, mm2_mode, debug_pv)
    if key not in _PROGRAM_CACHE:
        _PROGRAM_CACHE[key] = _build_program(mode, mm2_mode, debug_pv)
    return _PROGRAM_CACHE[key]


def kernel(reports_ids, PMI, W_lang, corr, W1, b1, W2, b2,
           _mode=None, _mm2_mode=None, _trace=False, _trace_out=None,
           _debug_pv=False):
    mode = MODE if _mode is None else _mode
    mm2_mode = MM2_MODE if _mm2_mode is None else _mm2_mode
    _, np_dt = _mode_dtypes(mode)
    mm2_np = ml_dtypes.bfloat16 if mm2_mode == "bf16" else np.float32
    two_pass = mode == "bf16x2"

    ids = np.asarray(reports_ids)
    PMI = np.asarray(PMI, dtype=np.float32)
    W_lang = np.asarray(W_lang, dtype=np.float32)
    corr = np.asarray(corr, dtype=np.float32)
    W1 = np.asarray(W1, dtype=np.float32)
    b1 = np.asarray(b1, dtype=np.float32)
    W2 = np.asarray(W2, dtype=np.float32)
    b2 = np.asarray(b2, dtype=np.float32)

    # host: exact integer token counts (the bag-of-words numerator)
    mask = (ids >= 0) & (ids < V)
    idx = np.clip(ids, 0, V - 1).astype(np.int64)
    counts = np.zeros((B, V), np.float32)
    np.add.at(counts, (np.broadcast_to(np.arange(B)[:, None], ids.shape), idx),
              mask.astype(np.float32))
    denom = np.maximum(counts.sum(axis=1), 1.0).astype(np.float32)
    bowT = np.ascontiguousarray(counts.T).astype(np_dt)          # [V, B]

    if two_pass:
        pmi_hi = PMI.astype(np_dt)
        pmi_lo = (PMI - pmi_hi.astype(np.float32)).astype(np_dt)
    else:
        pmi_hi = PMI.astype(np_dt)
        pmi_lo = None

    w1T = np.ascontiguousarray(W1.T)                             # [196, 512]
    corrT = np.ascontiguousarray(corr.reshape(-1)[:, None])      # [196, 1]
    b1c = np.ascontiguousarray(b1.reshape(H // 128, 128).T)      # [128, 4]
    w2T = np.ascontiguousarray(W2.T)                             # [512, 256]
    b2c = np.ascontiguousarray(b2.reshape(ZC // 128, 128).T)     # [128, 2]

    in_maps = []
    for c in range(NCORES):
        sl = slice(c * VS, (c + 1) * VS)
        m = {
            "pmi_hi": np.ascontiguousarray(pmi_hi[:, sl]),
            "bowT": bowT,
            "wlT": np.ascontiguousarray(W_lang[:, sl].T).astype(mm2_np),
            "w1T": w1T, "corrT": corrT, "b1c": b1c,
            "w2T": w2T, "b2c": b2c,
        }
        if two_pass:
            m["pmi_lo"] = np.ascontiguousarray(pmi_lo[:, sl])
        in_maps.append(m)

    nc = _get_program(mode, mm2_mode, _debug_pv)
    res = run_bass_kernel_spmd(nc, in_maps, core_ids=list(range(NCORES)),
                               trace=_trace)
    if _trace_out is not None:
        _trace_out["exec_time_ns"] = res.exec_time_ns
        _trace_out["results"] = res

    z_lang = np.zeros((B, ZL), np.float64)
    for c in range(NCORES):
        z_lang += res.results[c]["zpart"].astype(np.float64)
    z_lang = (z_lang / denom[:, None]).astype(np.float32)
    zc = res.results[0]["zcT"].reshape(ZC)
    out = np.empty((B, ZL + ZC), np.float32)
    out[:, :ZL] = z_lang
    out[:, ZL:] = zc[None, :]
    return out
